# revision 18
# baseline (speedup 1.0000x reference)
"""Llama4-style MoE experts (grouped SwiGLU MLP) on Trainium2, 8 NeuronCores.

Expert-parallel: core i runs expert i's full MLP on its 1024-token slice:
    out = (up * silu(gate)) @ W2,  [gate|up] = h @ W1
Per-core shapes: h [1024, 2048], W1 [2048, 8192], W2 [4096, 2048].

Matmuls run in bf16 on the TensorEngine.  All weight traffic uses gpsimd
SWDGE *casting* DMAs (f32 HBM -> bf16 SBUF directly), so there is no
f32 staging and no DVE cast pipeline: the PE waits only on DMA-complete
semaphores.  The W1 bf16 stripe ring holds two full 512-wide f-blocks,
so block fb+1 streams in while fb computes with zero boundary stalls.

h loads are split across the sync and vector HWDGE queues (4 rows
each); rows are PE-transposed (f32, via identity) as they land, and
fb0's matmuls run tb-major with i-pair PSUM groups so mm1 starts once
the first 4 rows have landed, with rows 4-7 transposed in the gaps.

mm2 runs f-outer / tt-inner with all 8 PSUM banks accumulating; W2
stripes trickle in ~12 f-tiles ahead of consumption.  Output drains
alternate the Scalar and Vector engines (copy + per-engine DMA queue),
and the last h-block's f-tail is processed per-tt so the drain
staggers into a short tail.
"""

from contextlib import ExitStack

import numpy as np

import concourse.bass as bass
import concourse.mybir as mybir
import concourse.tile as tile
from concourse import bacc
from concourse.bass_utils import run_bass_kernel_spmd
from concourse.masks import make_identity

N_CORES = 8
P = 128
TB = 512  # moving-operand free-dim block (one PSUM bank of f32)

F32 = mybir.dt.float32
BF16 = mybir.dt.bfloat16
ACT_SILU = mybir.ActivationFunctionType.Silu
ACT_COPY = mybir.ActivationFunctionType.Copy

# Per-core problem dims (full problem: 8 experts x 1024 tokens, H=2048, F=4096)
T = 1024
H = 2048
F = 4096


def build_kernel_body(tc, T=T, H=H, F=F):
    nc = tc.nc
    h_d = nc.dram_tensor("hidden_states", [T, H], F32, kind="ExternalInput").ap()
    w1_d = nc.dram_tensor("gate_up_proj", [H, 2 * F], F32, kind="ExternalInput").ap()
    w2_d = nc.dram_tensor("down_proj", [F, H], F32, kind="ExternalInput").ap()
    out_d = nc.dram_tensor("out", [T, H], F32, kind="ExternalOutput").ap()

    n_ht = H // P          # h-tiles (contraction tiles of matmul 1)
    n_ft = F // P          # f-tiles (rows of act; contraction tiles of matmul 2)
    n_tt = T // P          # token tiles (psum partition tiles of matmul 2)
    n_tb = T // TB         # token free-dim blocks in matmul 1
    n_fb = F // TB         # 512-wide f blocks of W1 (per gate/up half)
    n_hb = H // TB         # 512-wide h blocks of W2

    with ExitStack() as ctx:
        const = ctx.enter_context(tc.tile_pool(name="const", bufs=1))
        hcolp = ctx.enter_context(tc.tile_pool(name="hcolp", bufs=4))
        htp = ctx.enter_context(tc.tile_pool(name="htp", bufs=n_ht))
        actp = ctx.enter_context(tc.tile_pool(name="actp", bufs=n_ft))
        w1p = ctx.enter_context(tc.tile_pool(name="w1p", bufs=62))
        b2p = ctx.enter_context(tc.tile_pool(name="b2p", bufs=12))
        silp = ctx.enter_context(tc.tile_pool(name="silp", bufs=2))
        outp = ctx.enter_context(tc.tile_pool(name="outp", bufs=8))
        ps = ctx.enter_context(tc.tile_pool(name="ps", bufs=8, space="PSUM"))

        ident = const.tile([P, P], F32, tag="ident", name="ident")
        make_identity(nc, ident)

        ht = [htp.tile([P, T], BF16, tag="ht", name=f"ht{i}") for i in range(n_ht)]
        act = [actp.tile([P, T], BF16, tag="act", name=f"act{i}") for i in range(n_ft)]

        # ---- h rows: half-row f32 tiles, halves on both HWDGE queues ----
        # 4 half-row buffers mean a new row's DMA only WAR-waits a row
        # two back (already transposed) -- no pipeline serialization.
        hrowh = {}

        def dma_h_row(ti):
            for c in range(2):
                hr = hcolp.tile([P, H // 2], F32, tag="hrow",
                                name=f"hrow{ti}_{c}")
                eng = nc.sync if c == 0 else nc.scalar
                eng.dma_start(
                    hr[:],
                    h_d[ti * P : (ti + 1) * P,
                        c * (H // 2) : (c + 1) * (H // 2)],
                )
                hrowh[(ti, c)] = hr

        def transpose_row(ti):
            """PE-transpose one 128-token row block into ht (f32 -> bf16)."""
            for hh in range(n_ht):
                hr = hrowh[(ti, hh // 8)]
                col = (hh % 8) * P
                pt = ps.tile([P, TB], F32, tag="ps", name=f"tp{ti}_{hh}")
                nc.tensor.transpose(pt[:, :P], hr[:, col : col + P], ident)
                nc.vector.tensor_copy(
                    out=ht[hh][:, ti * P : (ti + 1) * P], in_=pt[:, :P]
                )

        # ---- W1: gpsimd casting DMAs, f32 HBM -> bf16 SBUF stripes ----
        w1g, w1u = {}, {}

        def dma_w1_block(fb):
            c0 = fb * TB
            for hh in range(n_ht):
                g = w1p.tile([P, TB], BF16, tag="w1", name=f"w1g_{fb}_{hh}")
                nc.gpsimd.dma_start(
                    out=g[:], in_=w1_d[hh * P : (hh + 1) * P, c0 : c0 + TB]
                )
                u = w1p.tile([P, TB], BF16, tag="w1", name=f"w1u_{fb}_{hh}")
                nc.gpsimd.dma_start(
                    out=u[:], in_=w1_d[hh * P : (hh + 1) * P, F + c0 : F + c0 + TB]
                )
                w1g[(fb, hh)] = g
                w1u[(fb, hh)] = u

        def swiglu(fi, tb, pg, pu):
            sg = silp.tile([P, TB], BF16, tag="silp", name=f"sig{fi}_{tb}")
            nc.scalar.activation(sg[:], pg[:], ACT_SILU)
            nc.vector.tensor_mul(
                out=act[fi][:, tb * TB : (tb + 1) * TB], in0=pu[:], in1=sg[:]
            )

        # ---- W2: gpsimd casting DMAs straight into the b2 ring ----
        b2 = {}

        def w2_stripe(hb, f):
            b = b2p.tile([P, TB], BF16, tag="b2", name=f"b2_{hb}_{f}")
            nc.gpsimd.dma_start(
                out=b[:], in_=w2_d[f * P : (f + 1) * P, hb * TB : (hb + 1) * TB]
            )
            b2[(hb, f)] = b

        # ---- Phase A/B0: h loads + transposes interleaved with fb0 ----
        # h rows stream on the two HWDGE queues in parallel with the W1
        # casting-DMA stream on the (single) SWDGE queue.
        for ti in range(8):
            dma_h_row(ti)
        dma_w1_block(0)
        for ti in range(4):
            transpose_row(ti)

        def sweep_fb0(tb, iset):
            """h-outer sweep over one tb-half of fb0 for an i-pair.

            Paces off the W1 DMA stream: matmul (i, hh) waits only on the
            gpsimd casting DMA of stripe (0, hh).
            """
            pg = {i: ps.tile([P, TB], F32, tag="ps", name=f"pg0_{i}_{tb}")
                  for i in iset}
            pu = {i: ps.tile([P, TB], F32, tag="ps", name=f"pu0_{i}_{tb}")
                  for i in iset}
            for hh in range(n_ht):
                first, last = hh == 0, hh == n_ht - 1
                for i in iset:
                    for p, w in ((pg, w1g), (pu, w1u)):
                        nc.tensor.matmul(
                            p[i][:],
                            lhsT=w[(0, hh)][:, i * P : (i + 1) * P],
                            rhs=ht[hh][:, tb * TB : (tb + 1) * TB],
                            start=first,
                            stop=last,
                        )
            for i in iset:
                swiglu(i, tb, pg[i], pu[i])

        # rows 4-7 transpose between sweeps (they land while sweep A runs;
        # no psum accumulation groups are held at these points).  fb1's
        # stripes are issued after sweep A so startup DMA bandwidth goes
        # to h and fb0 first.
        sweep_fb0(0, (0, 1))
        dma_w1_block(1)
        transpose_row(4)
        transpose_row(5)
        sweep_fb0(0, (2, 3))
        transpose_row(6)
        transpose_row(7)
        sweep_fb0(1, (0, 1))
        sweep_fb0(1, (2, 3))

        # ---- Phase B: steady-state f-blocks 1..n_fb-1 ----
        for fb in range(1, n_fb):
            if fb + 1 < n_fb:
                dma_w1_block(fb + 1)
            for i in range(TB // P):
                fi = fb * (TB // P) + i
                pg = {tb: ps.tile([P, TB], F32, tag="ps", name=f"pg{fi}_{tb}")
                      for tb in range(n_tb)}
                pu = {tb: ps.tile([P, TB], F32, tag="ps", name=f"pu{fi}_{tb}")
                      for tb in range(n_tb)}
                for hh in range(n_ht):
                    first, last = hh == 0, hh == n_ht - 1
                    for p, w in ((pg, w1g), (pu, w1u)):
                        for tb in range(n_tb):
                            nc.tensor.matmul(
                                p[tb][:],
                                lhsT=w[(fb, hh)][:, i * P : (i + 1) * P],
                                rhs=ht[hh][:, tb * TB : (tb + 1) * TB],
                                start=first,
                                stop=last,
                            )
                for tb in range(n_tb):
                    swiglu(fi, tb, pg[tb], pu[tb])
            if fb == n_fb - 2:
                # W2 hb0 prefetch (capped at the b2 ring depth)
                for f in range(12):
                    w2_stripe(0, f)

        # ---- Phase C: out = act @ W2, f-outer / tt-inner, 8 psum banks ----
        def drain(hb, tt, po):
            ob = outp.tile([P, TB], F32, tag="outp", name=f"ob{hb}_{tt}")
            if tt % 2 == 0:
                nc.scalar.activation(ob[:], po[:], ACT_COPY)
                nc.scalar.dma_start(
                    out_d[tt * P : (tt + 1) * P, hb * TB : (hb + 1) * TB], ob[:]
                )
            else:
                nc.vector.tensor_copy(out=ob[:], in_=po[:])
                nc.sync.dma_start(
                    out_d[tt * P : (tt + 1) * P, hb * TB : (hb + 1) * TB], ob[:]
                )

        # strict FIFO stripe schedule: allocation runs exactly AHEAD of
        # consumption so the b2 ring can never form a cross-phase cycle
        SCHED = [(hb, f) for hb in range(n_hb) for f in range(n_ft)]
        AHEAD = 12

        def trickle(g):
            if g + AHEAD < len(SCHED):
                w2_stripe(*SCHED[g + AHEAD])

        FTAIL = 4  # last-hb f-tiles processed per-tt so the drain staggers
        for hb in range(n_hb):
            last_hb = hb == n_hb - 1
            po = [ps.tile([P, TB], F32, tag="ps", name=f"po{hb}_{tt}")
                  for tt in range(n_tt)]
            n_f_main = n_ft - FTAIL if last_hb else n_ft
            for f in range(n_f_main):
                trickle(hb * n_ft + f)
                rhs = b2[(hb, f)][:]
                for tt in range(n_tt):
                    nc.tensor.matmul(
                        po[tt][:],
                        lhsT=act[f][:, tt * P : (tt + 1) * P],
                        rhs=rhs,
                        start=(f == 0),
                        stop=(not last_hb and f == n_ft - 1),
                    )
            if not last_hb:
                for tt in range(n_tt):
                    drain(hb, tt, po[tt])
            else:
                for f in range(n_f_main, n_ft):
                    trickle(hb * n_ft + f)
                for tt in range(n_tt):
                    for f in range(n_f_main, n_ft):
                        nc.tensor.matmul(
                            po[tt][:],
                            lhsT=act[f][:, tt * P : (tt + 1) * P],
                            rhs=b2[(hb, f)][:],
                            start=False,
                            stop=(f == n_ft - 1),
                        )
                    if tt < n_tt - 2:
                        drain(hb, tt, po[tt])
                    else:
                        # final drains split in half across scalar+vector so
                        # the copy and out-DMA tail overlap
                        ob = outp.tile([P, TB], F32, tag="outp",
                                       name=f"ob{hb}_{tt}")
                        hw = TB // 2
                        nc.scalar.activation(ob[:, :hw], po[tt][:, :hw],
                                             ACT_COPY)
                        nc.scalar.dma_start(
                            out_d[tt * P : (tt + 1) * P,
                                  hb * TB : hb * TB + hw],
                            ob[:, :hw],
                        )
                        nc.vector.tensor_copy(out=ob[:, hw:],
                                              in_=po[tt][:, hw:])
                        nc.sync.dma_start(
                            out_d[tt * P : (tt + 1) * P,
                                  hb * TB + hw : (hb + 1) * TB],
                            ob[:, hw:],
                        )


def build_nc(T=T, H=H, F=F):
    nc = bacc.Bacc(
        "TRN2", target_bir_lowering=False, debug=False, enable_asserts=False
    )
    with tile.TileContext(nc) as tc:
        build_kernel_body(tc, T=T, H=H, F=F)
    nc.compile()
    return nc


_NC_CACHE = None


def run(hidden_states, gate_up_proj, down_proj, trace=False, **kw):
    """Run on the 8 NeuronCores; returns (output, BassKernelResults)."""
    global _NC_CACHE
    if _NC_CACHE is None:
        _NC_CACHE = build_nc()
    nc = _NC_CACHE

    hs = np.ascontiguousarray(np.asarray(hidden_states), dtype=np.float32)
    gup = np.ascontiguousarray(np.asarray(gate_up_proj), dtype=np.float32)
    dp = np.ascontiguousarray(np.asarray(down_proj), dtype=np.float32)
    assert hs.shape == (N_CORES * T, H), hs.shape
    assert gup.shape == (N_CORES, H, 2 * F), gup.shape
    assert dp.shape == (N_CORES, F, H), dp.shape

    in_maps = [
        {
            "hidden_states": np.ascontiguousarray(hs[i * T : (i + 1) * T]),
            "gate_up_proj": np.ascontiguousarray(gup[i]),
            "down_proj": np.ascontiguousarray(dp[i]),
        }
        for i in range(N_CORES)
    ]
    res = run_bass_kernel_spmd(
        nc, in_maps, core_ids=list(range(N_CORES)), trace=trace, **kw
    )
    out = np.concatenate(
        [res.results[i]["out"] for i in range(N_CORES)], axis=0
    ).astype(np.float32)
    return out, res


def kernel(hidden_states, gate_up_proj, down_proj):
    out, _ = run(hidden_states, gate_up_proj, down_proj, trace=False)
    return out


# revision 22
# speedup vs baseline: 1.0039x; 1.0039x over previous
"""Llama4-style MoE experts (grouped SwiGLU MLP) on Trainium2, 8 NeuronCores.

Expert-parallel: core i runs expert i's full MLP on its 1024-token slice:
    out = (up * silu(gate)) @ W2,  [gate|up] = h @ W1
Per-core shapes: h [1024, 2048], W1 [2048, 8192], W2 [4096, 2048].

Matmuls run in bf16 on the TensorEngine.  All weight traffic uses gpsimd
SWDGE *casting* DMAs (f32 HBM -> bf16 SBUF directly), so there is no
f32 staging and no DVE cast pipeline: the PE waits only on DMA-complete
semaphores.  The W1 bf16 stripe ring holds two full 512-wide f-blocks,
so block fb+1 streams in while fb computes with zero boundary stalls.

h loads are split across the sync and vector HWDGE queues (4 rows
each); rows are PE-transposed (f32, via identity) as they land, and
fb0's matmuls run tb-major with i-pair PSUM groups so mm1 starts once
the first 4 rows have landed, with rows 4-7 transposed in the gaps.

mm2 runs f-outer / tt-inner with all 8 PSUM banks accumulating; W2
stripes trickle in ~12 f-tiles ahead of consumption.  Output drains
alternate the Scalar and Vector engines (copy + per-engine DMA queue),
and the last h-block's f-tail is processed per-tt so the drain
staggers into a short tail.
"""

from contextlib import ExitStack

import numpy as np

import concourse.bass as bass
import concourse.mybir as mybir
import concourse.tile as tile
from concourse import bacc
from concourse.bass_utils import run_bass_kernel_spmd
from concourse.masks import make_identity

N_CORES = 8
P = 128
TB = 512  # moving-operand free-dim block (one PSUM bank of f32)

F32 = mybir.dt.float32
BF16 = mybir.dt.bfloat16
ACT_SILU = mybir.ActivationFunctionType.Silu
ACT_COPY = mybir.ActivationFunctionType.Copy

# Per-core problem dims (full problem: 8 experts x 1024 tokens, H=2048, F=4096)
T = 1024
H = 2048
F = 4096


def build_kernel_body(tc, T=T, H=H, F=F):
    nc = tc.nc
    h_d = nc.dram_tensor("hidden_states", [T, H], F32, kind="ExternalInput").ap()
    w1_d = nc.dram_tensor("gate_up_proj", [H, 2 * F], F32, kind="ExternalInput").ap()
    w2_d = nc.dram_tensor("down_proj", [F, H], F32, kind="ExternalInput").ap()
    out_d = nc.dram_tensor("out", [T, H], F32, kind="ExternalOutput").ap()

    n_ht = H // P          # h-tiles (contraction tiles of matmul 1)
    n_ft = F // P          # f-tiles (rows of act; contraction tiles of matmul 2)
    n_tt = T // P          # token tiles (psum partition tiles of matmul 2)
    n_tb = T // TB         # token free-dim blocks in matmul 1
    n_fb = F // TB         # 512-wide f blocks of W1 (per gate/up half)
    n_hb = H // TB         # 512-wide h blocks of W2

    with ExitStack() as ctx:
        const = ctx.enter_context(tc.tile_pool(name="const", bufs=1))
        hcolp = ctx.enter_context(tc.tile_pool(name="hcolp", bufs=4))
        hbfp = ctx.enter_context(tc.tile_pool(name="hbfp", bufs=3))
        htp = ctx.enter_context(tc.tile_pool(name="htp", bufs=n_ht))
        actp = ctx.enter_context(tc.tile_pool(name="actp", bufs=n_ft))
        w1p = ctx.enter_context(tc.tile_pool(name="w1p", bufs=56))
        b2p = ctx.enter_context(tc.tile_pool(name="b2p", bufs=8))
        silp = ctx.enter_context(tc.tile_pool(name="silp", bufs=2))
        outp = ctx.enter_context(tc.tile_pool(name="outp", bufs=8))
        ps = ctx.enter_context(tc.tile_pool(name="ps", bufs=8, space="PSUM"))

        identb = const.tile([P, P], BF16, tag="identb", name="identb")
        make_identity(nc, identb)

        ht = [htp.tile([P, T], BF16, tag="ht", name=f"ht{i}") for i in range(n_ht)]
        act = [actp.tile([P, T], BF16, tag="act", name=f"act{i}") for i in range(n_ft)]

        # ---- h rows: half-row f32 DMAs on both HWDGE queues, cast to
        # bf16 rows on the Scalar engine, then bf16 PE transposes (1
        # cycle/row vs f32's 2). 4 half-row staging buffers mean a new
        # row's DMA only WAR-waits its scalar cast -- no serialization.
        hrowh = {}
        hrow = {}

        def dma_h_row(ti):
            for c in range(2):
                hr = hcolp.tile([P, H // 2], F32, tag="hrow",
                                name=f"hrow{ti}_{c}")
                eng = nc.sync if c == 0 else nc.scalar
                eng.dma_start(
                    hr[:],
                    h_d[ti * P : (ti + 1) * P,
                        c * (H // 2) : (c + 1) * (H // 2)],
                )
                hrowh[(ti, c)] = hr

        def cast_row(ti):
            hb = hbfp.tile([P, H], BF16, tag="hb", name=f"hb{ti}")
            for c in range(2):
                nc.scalar.activation(
                    hb[:, c * (H // 2) : (c + 1) * (H // 2)],
                    hrowh[(ti, c)][:], ACT_COPY,
                )
            hrow[ti] = hb

        def transpose_row(ti):
            """PE-transpose one bf16 128-token row block into ht."""
            hr = hrow[ti]
            for hh in range(n_ht):
                pt = ps.tile([P, TB], F32, tag="ps", name=f"tp{ti}_{hh}")
                ptb = pt[:].bitcast(BF16)
                nc.tensor.transpose(
                    ptb[:, :P], hr[:, hh * P : (hh + 1) * P], identb
                )
                nc.vector.tensor_copy(
                    out=ht[hh][:, ti * P : (ti + 1) * P], in_=ptb[:, :P]
                )

        # ---- W1: gpsimd casting DMAs, f32 HBM -> bf16 SBUF stripes ----
        w1g, w1u = {}, {}

        def dma_w1_block(fb):
            c0 = fb * TB
            for hh in range(n_ht):
                g = w1p.tile([P, TB], BF16, tag="w1", name=f"w1g_{fb}_{hh}")
                nc.gpsimd.dma_start(
                    out=g[:], in_=w1_d[hh * P : (hh + 1) * P, c0 : c0 + TB]
                )
                u = w1p.tile([P, TB], BF16, tag="w1", name=f"w1u_{fb}_{hh}")
                nc.gpsimd.dma_start(
                    out=u[:], in_=w1_d[hh * P : (hh + 1) * P, F + c0 : F + c0 + TB]
                )
                w1g[(fb, hh)] = g
                w1u[(fb, hh)] = u

        def swiglu(fi, tb, pg, pu):
            sg = silp.tile([P, TB], BF16, tag="silp", name=f"sig{fi}_{tb}")
            nc.scalar.activation(sg[:], pg[:], ACT_SILU)
            nc.vector.tensor_mul(
                out=act[fi][:, tb * TB : (tb + 1) * TB], in0=pu[:], in1=sg[:]
            )

        # ---- W2: gpsimd casting DMAs straight into the b2 ring ----
        b2 = {}

        def w2_stripe(hb, f):
            b = b2p.tile([P, TB], BF16, tag="b2", name=f"b2_{hb}_{f}")
            nc.gpsimd.dma_start(
                out=b[:], in_=w2_d[f * P : (f + 1) * P, hb * TB : (hb + 1) * TB]
            )
            b2[(hb, f)] = b

        # ---- Phase A/B0: h loads + transposes interleaved with fb0 ----
        # h rows stream on the two HWDGE queues in parallel with the W1
        # casting-DMA stream on the (single) SWDGE queue.
        for ti in range(8):
            dma_h_row(ti)
        dma_w1_block(0)
        for ti in range(4):
            cast_row(ti)
            transpose_row(ti)
        # rows 4-6 cast eagerly (hbfp WAR reaches rows 1-3, transposed
        # above); row 7's cast must trail transpose_row(4) or it blocks
        # the scalar queue ahead of sweep A's silu
        cast_row(4)
        cast_row(5)
        cast_row(6)

        def sweep_fb0(tb, iset):
            """h-outer sweep over one tb-half of fb0 for an i-pair.

            Paces off the W1 DMA stream: matmul (i, hh) waits only on the
            gpsimd casting DMA of stripe (0, hh).
            """
            pg = {i: ps.tile([P, TB], F32, tag="ps", name=f"pg0_{i}_{tb}")
                  for i in iset}
            pu = {i: ps.tile([P, TB], F32, tag="ps", name=f"pu0_{i}_{tb}")
                  for i in iset}
            for hh in range(n_ht):
                first, last = hh == 0, hh == n_ht - 1
                for i in iset:
                    for p, w in ((pg, w1g), (pu, w1u)):
                        nc.tensor.matmul(
                            p[i][:],
                            lhsT=w[(0, hh)][:, i * P : (i + 1) * P],
                            rhs=ht[hh][:, tb * TB : (tb + 1) * TB],
                            start=first,
                            stop=last,
                        )
            for i in iset:
                swiglu(i, tb, pg[i], pu[i])

        # rows 4-7 transpose between sweeps (they land while sweep A runs;
        # no psum accumulation groups are held at these points).  fb1's
        # stripes are issued after sweep A so startup DMA bandwidth goes
        # to h and fb0 first.
        sweep_fb0(0, (0, 1))
        dma_w1_block(1)
        transpose_row(4)
        cast_row(7)
        transpose_row(5)
        sweep_fb0(0, (2, 3))
        transpose_row(6)
        transpose_row(7)
        sweep_fb0(1, (0, 1))
        sweep_fb0(1, (2, 3))

        # ---- Phase B: steady-state f-blocks 1..n_fb-1 ----
        for fb in range(1, n_fb):
            if fb + 1 < n_fb:
                dma_w1_block(fb + 1)
            for i in range(TB // P):
                fi = fb * (TB // P) + i
                pg = {tb: ps.tile([P, TB], F32, tag="ps", name=f"pg{fi}_{tb}")
                      for tb in range(n_tb)}
                pu = {tb: ps.tile([P, TB], F32, tag="ps", name=f"pu{fi}_{tb}")
                      for tb in range(n_tb)}
                for hh in range(n_ht):
                    first, last = hh == 0, hh == n_ht - 1
                    for p, w in ((pg, w1g), (pu, w1u)):
                        for tb in range(n_tb):
                            nc.tensor.matmul(
                                p[tb][:],
                                lhsT=w[(fb, hh)][:, i * P : (i + 1) * P],
                                rhs=ht[hh][:, tb * TB : (tb + 1) * TB],
                                start=first,
                                stop=last,
                            )
                for tb in range(n_tb):
                    swiglu(fi, tb, pg[tb], pu[tb])
            if fb == n_fb - 2:
                # W2 hb0 prefetch (capped at the b2 ring depth)
                for f in range(12):
                    w2_stripe(0, f)

        # ---- Phase C: out = act @ W2, f-outer / tt-inner, 8 psum banks ----
        def drain(hb, tt, po):
            ob = outp.tile([P, TB], F32, tag="outp", name=f"ob{hb}_{tt}")
            if tt % 2 == 0:
                nc.scalar.activation(ob[:], po[:], ACT_COPY)
                nc.scalar.dma_start(
                    out_d[tt * P : (tt + 1) * P, hb * TB : (hb + 1) * TB], ob[:]
                )
            else:
                nc.vector.tensor_copy(out=ob[:], in_=po[:])
                nc.sync.dma_start(
                    out_d[tt * P : (tt + 1) * P, hb * TB : (hb + 1) * TB], ob[:]
                )

        # strict FIFO stripe schedule: allocation runs exactly AHEAD of
        # consumption so the b2 ring can never form a cross-phase cycle
        SCHED = [(hb, f) for hb in range(n_hb) for f in range(n_ft)]
        AHEAD = 12

        def trickle(g):
            if g + AHEAD < len(SCHED):
                w2_stripe(*SCHED[g + AHEAD])

        FTAIL = 4  # last-hb f-tiles processed per-tt so the drain staggers
        for hb in range(n_hb):
            last_hb = hb == n_hb - 1
            po = [ps.tile([P, TB], F32, tag="ps", name=f"po{hb}_{tt}")
                  for tt in range(n_tt)]
            n_f_main = n_ft - FTAIL if last_hb else n_ft
            for f in range(n_f_main):
                trickle(hb * n_ft + f)
                rhs = b2[(hb, f)][:]
                for tt in range(n_tt):
                    nc.tensor.matmul(
                        po[tt][:],
                        lhsT=act[f][:, tt * P : (tt + 1) * P],
                        rhs=rhs,
                        start=(f == 0),
                        stop=(not last_hb and f == n_ft - 1),
                    )
            if not last_hb:
                for tt in range(n_tt):
                    drain(hb, tt, po[tt])
            else:
                for f in range(n_f_main, n_ft):
                    trickle(hb * n_ft + f)
                for tt in range(n_tt):
                    for f in range(n_f_main, n_ft):
                        nc.tensor.matmul(
                            po[tt][:],
                            lhsT=act[f][:, tt * P : (tt + 1) * P],
                            rhs=b2[(hb, f)][:],
                            start=False,
                            stop=(f == n_ft - 1),
                        )
                    if tt < n_tt - 2:
                        drain(hb, tt, po[tt])
                    else:
                        # final drains split in half across scalar+vector so
                        # the copy and out-DMA tail overlap
                        ob = outp.tile([P, TB], F32, tag="outp",
                                       name=f"ob{hb}_{tt}")
                        hw = TB // 2
                        nc.scalar.activation(ob[:, :hw], po[tt][:, :hw],
                                             ACT_COPY)
                        nc.scalar.dma_start(
                            out_d[tt * P : (tt + 1) * P,
                                  hb * TB : hb * TB + hw],
                            ob[:, :hw],
                        )
                        nc.vector.tensor_copy(out=ob[:, hw:],
                                              in_=po[tt][:, hw:])
                        nc.sync.dma_start(
                            out_d[tt * P : (tt + 1) * P,
                                  hb * TB + hw : (hb + 1) * TB],
                            ob[:, hw:],
                        )


def build_nc(T=T, H=H, F=F):
    nc = bacc.Bacc(
        "TRN2", target_bir_lowering=False, debug=False, enable_asserts=False
    )
    with tile.TileContext(nc) as tc:
        build_kernel_body(tc, T=T, H=H, F=F)
    nc.compile()
    return nc


_NC_CACHE = None


def run(hidden_states, gate_up_proj, down_proj, trace=False, **kw):
    """Run on the 8 NeuronCores; returns (output, BassKernelResults)."""
    global _NC_CACHE
    if _NC_CACHE is None:
        _NC_CACHE = build_nc()
    nc = _NC_CACHE

    hs = np.ascontiguousarray(np.asarray(hidden_states), dtype=np.float32)
    gup = np.ascontiguousarray(np.asarray(gate_up_proj), dtype=np.float32)
    dp = np.ascontiguousarray(np.asarray(down_proj), dtype=np.float32)
    assert hs.shape == (N_CORES * T, H), hs.shape
    assert gup.shape == (N_CORES, H, 2 * F), gup.shape
    assert dp.shape == (N_CORES, F, H), dp.shape

    in_maps = [
        {
            "hidden_states": np.ascontiguousarray(hs[i * T : (i + 1) * T]),
            "gate_up_proj": np.ascontiguousarray(gup[i]),
            "down_proj": np.ascontiguousarray(dp[i]),
        }
        for i in range(N_CORES)
    ]
    res = run_bass_kernel_spmd(
        nc, in_maps, core_ids=list(range(N_CORES)), trace=trace, **kw
    )
    out = np.concatenate(
        [res.results[i]["out"] for i in range(N_CORES)], axis=0
    ).astype(np.float32)
    return out, res


def kernel(hidden_states, gate_up_proj, down_proj):
    out, _ = run(hidden_states, gate_up_proj, down_proj, trace=False)
    return out


# revision 27
# speedup vs baseline: 1.0062x; 1.0023x over previous
"""Llama4-style MoE experts (grouped SwiGLU MLP) on Trainium2, 8 NeuronCores.

Expert-parallel: core i runs expert i's full MLP on its 1024-token slice:
    out = (up * silu(gate)) @ W2,  [gate|up] = h @ W1
Per-core shapes: h [1024, 2048], W1 [2048, 8192], W2 [4096, 2048].

Matmuls run in bf16 on the TensorEngine.  All weight traffic uses gpsimd
SWDGE *casting* DMAs (f32 HBM -> bf16 SBUF directly), so there is no
f32 staging and no DVE cast pipeline: the PE waits only on DMA-complete
semaphores.  The W1 bf16 stripe ring holds two full 512-wide f-blocks,
so block fb+1 streams in while fb computes with zero boundary stalls.

h loads are split across the sync and vector HWDGE queues (4 rows
each); rows are PE-transposed (f32, via identity) as they land, and
fb0's matmuls run tb-major with i-pair PSUM groups so mm1 starts once
the first 4 rows have landed, with rows 4-7 transposed in the gaps.

mm2 runs f-outer / tt-inner with all 8 PSUM banks accumulating; W2
stripes trickle in ~12 f-tiles ahead of consumption.  Output drains
alternate the Scalar and Vector engines (copy + per-engine DMA queue),
and the last h-block's f-tail is processed per-tt so the drain
staggers into a short tail.
"""

from contextlib import ExitStack

import numpy as np

import concourse.bass as bass
import concourse.mybir as mybir
import concourse.tile as tile
from concourse import bacc
from concourse.bass_utils import run_bass_kernel_spmd
from concourse.masks import make_identity

N_CORES = 8
P = 128
TB = 512  # moving-operand free-dim block (one PSUM bank of f32)

F32 = mybir.dt.float32
BF16 = mybir.dt.bfloat16
ACT_SILU = mybir.ActivationFunctionType.Silu
ACT_COPY = mybir.ActivationFunctionType.Copy

# Per-core problem dims (full problem: 8 experts x 1024 tokens, H=2048, F=4096)
T = 1024
H = 2048
F = 4096


def build_kernel_body(tc, T=T, H=H, F=F):
    nc = tc.nc
    h_d = nc.dram_tensor("hidden_states", [T, H], F32, kind="ExternalInput").ap()
    w1_d = nc.dram_tensor("gate_up_proj", [H, 2 * F], F32, kind="ExternalInput").ap()
    w2_d = nc.dram_tensor("down_proj", [F, H], F32, kind="ExternalInput").ap()
    out_d = nc.dram_tensor("out", [T, H], F32, kind="ExternalOutput").ap()

    n_ht = H // P          # h-tiles (contraction tiles of matmul 1)
    n_ft = F // P          # f-tiles (rows of act; contraction tiles of matmul 2)
    n_tt = T // P          # token tiles (psum partition tiles of matmul 2)
    n_tb = T // TB         # token free-dim blocks in matmul 1
    n_fb = F // TB         # 512-wide f blocks of W1 (per gate/up half)
    n_hb = H // TB         # 512-wide h blocks of W2

    with ExitStack() as ctx:
        const = ctx.enter_context(tc.tile_pool(name="const", bufs=1))
        hbfp = ctx.enter_context(tc.tile_pool(name="hbfp", bufs=4))
        htp = ctx.enter_context(tc.tile_pool(name="htp", bufs=n_ht))
        actp = ctx.enter_context(tc.tile_pool(name="actp", bufs=n_ft))
        w1p = ctx.enter_context(tc.tile_pool(name="w1p", bufs=62))
        b2p = ctx.enter_context(tc.tile_pool(name="b2p", bufs=12))
        silp = ctx.enter_context(tc.tile_pool(name="silp", bufs=2))
        outp = ctx.enter_context(tc.tile_pool(name="outp", bufs=8))
        ps = ctx.enter_context(tc.tile_pool(name="ps", bufs=8, space="PSUM"))

        identb = const.tile([P, P], BF16, tag="identb", name="identb")
        make_identity(nc, identb)

        ht = [htp.tile([P, T], BF16, tag="ht", name=f"ht{i}") for i in range(n_ht)]
        act = [actp.tile([P, T], BF16, tag="act", name=f"act{i}") for i in range(n_ft)]

        # ---- h rows: gpsimd casting DMAs (f32 HBM -> bf16 SBUF) ----
        hrow = {}

        def dma_h_row(ti):
            hr = hbfp.tile([P, H], BF16, tag="hrow", name=f"hrow{ti}")
            nc.gpsimd.dma_start(out=hr[:], in_=h_d[ti * P : (ti + 1) * P, :])
            hrow[ti] = hr

        def transpose_row(ti):
            """PE-transpose one bf16 128-token row block into ht."""
            hr = hrow[ti]
            for hh in range(n_ht):
                pt = ps.tile([P, TB], F32, tag="ps", name=f"tp{ti}_{hh}")
                ptb = pt[:].bitcast(BF16)
                nc.tensor.transpose(
                    ptb[:, :P], hr[:, hh * P : (hh + 1) * P], identb
                )
                nc.vector.tensor_copy(
                    out=ht[hh][:, ti * P : (ti + 1) * P], in_=ptb[:, :P]
                )

        def warmup_pe(n=32):
            """Junk identity matmuls that fill the pre-row0 dead window so
            the PE is at max p-state when the first transpose arrives."""
            for k in range(n):
                wt = ps.tile([P, TB], F32, tag="ps", name=f"wu{k}")
                nc.tensor.matmul(
                    wt[:, :P], lhsT=identb[:], rhs=identb[:],
                    start=True, stop=True,
                )

        # ---- W1: gpsimd casting DMAs, f32 HBM -> bf16 SBUF stripes ----
        w1g, w1u = {}, {}

        def dma_w1_block(fb):
            c0 = fb * TB
            for hh in range(n_ht):
                g = w1p.tile([P, TB], BF16, tag="w1", name=f"w1g_{fb}_{hh}")
                nc.gpsimd.dma_start(
                    out=g[:], in_=w1_d[hh * P : (hh + 1) * P, c0 : c0 + TB]
                )
                u = w1p.tile([P, TB], BF16, tag="w1", name=f"w1u_{fb}_{hh}")
                nc.gpsimd.dma_start(
                    out=u[:], in_=w1_d[hh * P : (hh + 1) * P, F + c0 : F + c0 + TB]
                )
                w1g[(fb, hh)] = g
                w1u[(fb, hh)] = u

        def swiglu(fi, tb, pg, pu):
            sg = silp.tile([P, TB], BF16, tag="silp", name=f"sig{fi}_{tb}")
            nc.scalar.activation(sg[:], pg[:], ACT_SILU)
            nc.vector.tensor_mul(
                out=act[fi][:, tb * TB : (tb + 1) * TB], in0=pu[:], in1=sg[:]
            )

        # ---- W2: gpsimd casting DMAs straight into the b2 ring ----
        b2 = {}

        def w2_stripe(hb, f):
            b = b2p.tile([P, TB], BF16, tag="b2", name=f"b2_{hb}_{f}")
            nc.gpsimd.dma_start(
                out=b[:], in_=w2_d[f * P : (f + 1) * P, hb * TB : (hb + 1) * TB]
            )
            b2[(hb, f)] = b

        # ---- Phase A/B0: h loads + transposes interleaved with fb0 ----
        # gpsimd issue order: rows 0-3, all of fb0, rows 4-7 (the late
        # rows WAR-wait on rows 0-3's transposes, so they must trail fb0
        # to avoid head-of-line blocking the W1 stream).
        for ti in range(4):
            dma_h_row(ti)
        dma_w1_block(0)
        for ti in range(4, 8):
            dma_h_row(ti)
        warmup_pe()
        for ti in range(4):
            transpose_row(ti)

        def sweep_fb0(tb, iset):
            """h-outer sweep over one tb-half of fb0 for an i-pair.

            Paces off the W1 DMA stream: matmul (i, hh) waits only on the
            gpsimd casting DMA of stripe (0, hh).
            """
            pg = {i: ps.tile([P, TB], F32, tag="ps", name=f"pg0_{i}_{tb}")
                  for i in iset}
            pu = {i: ps.tile([P, TB], F32, tag="ps", name=f"pu0_{i}_{tb}")
                  for i in iset}
            for hh in range(n_ht):
                first, last = hh == 0, hh == n_ht - 1
                for i in iset:
                    for p, w in ((pg, w1g), (pu, w1u)):
                        nc.tensor.matmul(
                            p[i][:],
                            lhsT=w[(0, hh)][:, i * P : (i + 1) * P],
                            rhs=ht[hh][:, tb * TB : (tb + 1) * TB],
                            start=first,
                            stop=last,
                        )
            for i in iset:
                swiglu(i, tb, pg[i], pu[i])

        # rows 4-7 transpose between sweeps (they land while sweep A runs;
        # no psum accumulation groups are held at these points).  fb1's
        # stripes are issued after sweep A so startup DMA bandwidth goes
        # to h and fb0 first.
        sweep_fb0(0, (0, 1))
        dma_w1_block(1)
        transpose_row(4)
        transpose_row(5)
        sweep_fb0(0, (2, 3))
        transpose_row(6)
        transpose_row(7)
        sweep_fb0(1, (0, 1))
        sweep_fb0(1, (2, 3))

        # ---- Phase B: steady-state f-blocks 1..n_fb-1 ----
        for fb in range(1, n_fb):
            if fb + 1 < n_fb:
                dma_w1_block(fb + 1)
            for i in range(TB // P):
                fi = fb * (TB // P) + i
                pg = {tb: ps.tile([P, TB], F32, tag="ps", name=f"pg{fi}_{tb}")
                      for tb in range(n_tb)}
                pu = {tb: ps.tile([P, TB], F32, tag="ps", name=f"pu{fi}_{tb}")
                      for tb in range(n_tb)}
                for hh in range(n_ht):
                    first, last = hh == 0, hh == n_ht - 1
                    for p, w in ((pg, w1g), (pu, w1u)):
                        for tb in range(n_tb):
                            nc.tensor.matmul(
                                p[tb][:],
                                lhsT=w[(fb, hh)][:, i * P : (i + 1) * P],
                                rhs=ht[hh][:, tb * TB : (tb + 1) * TB],
                                start=first,
                                stop=last,
                            )
                for tb in range(n_tb):
                    swiglu(fi, tb, pg[tb], pu[tb])
            if fb == n_fb - 2:
                # W2 hb0 prefetch (capped at the b2 ring depth)
                for f in range(12):
                    w2_stripe(0, f)

        # ---- Phase C: out = act @ W2, f-outer / tt-inner, 8 psum banks ----
        def drain(hb, tt, po):
            ob = outp.tile([P, TB], F32, tag="outp", name=f"ob{hb}_{tt}")
            if tt % 2 == 0:
                nc.scalar.activation(ob[:], po[:], ACT_COPY)
                nc.scalar.dma_start(
                    out_d[tt * P : (tt + 1) * P, hb * TB : (hb + 1) * TB], ob[:]
                )
            else:
                nc.vector.tensor_copy(out=ob[:], in_=po[:])
                nc.sync.dma_start(
                    out_d[tt * P : (tt + 1) * P, hb * TB : (hb + 1) * TB], ob[:]
                )

        # strict FIFO stripe schedule: allocation runs exactly AHEAD of
        # consumption so the b2 ring can never form a cross-phase cycle
        SCHED = [(hb, f) for hb in range(n_hb) for f in range(n_ft)]
        AHEAD = 12

        def trickle(g):
            if g + AHEAD < len(SCHED):
                w2_stripe(*SCHED[g + AHEAD])

        FTAIL = 4  # last-hb f-tiles processed per-tt so the drain staggers
        for hb in range(n_hb):
            last_hb = hb == n_hb - 1
            po = [ps.tile([P, TB], F32, tag="ps", name=f"po{hb}_{tt}")
                  for tt in range(n_tt)]
            n_f_main = n_ft - FTAIL if last_hb else n_ft
            for f in range(n_f_main):
                trickle(hb * n_ft + f)
                rhs = b2[(hb, f)][:]
                for tt in range(n_tt):
                    nc.tensor.matmul(
                        po[tt][:],
                        lhsT=act[f][:, tt * P : (tt + 1) * P],
                        rhs=rhs,
                        start=(f == 0),
                        stop=(not last_hb and f == n_ft - 1),
                    )
            if not last_hb:
                for tt in range(n_tt):
                    drain(hb, tt, po[tt])
            else:
                for f in range(n_f_main, n_ft):
                    trickle(hb * n_ft + f)
                for tt in range(n_tt):
                    for f in range(n_f_main, n_ft):
                        nc.tensor.matmul(
                            po[tt][:],
                            lhsT=act[f][:, tt * P : (tt + 1) * P],
                            rhs=b2[(hb, f)][:],
                            start=False,
                            stop=(f == n_ft - 1),
                        )
                    if tt < n_tt - 2:
                        drain(hb, tt, po[tt])
                    else:
                        # final drains split in quarters across scalar and
                        # vector so the copy and out-DMA tail overlap
                        ob = outp.tile([P, TB], F32, tag="outp",
                                       name=f"ob{hb}_{tt}")
                        qw = TB // 4
                        for q in range(4):
                            lo, hi = q * qw, (q + 1) * qw
                            if q % 2 == 0:
                                nc.scalar.activation(ob[:, lo:hi],
                                                     po[tt][:, lo:hi],
                                                     ACT_COPY)
                                nc.scalar.dma_start(
                                    out_d[tt * P : (tt + 1) * P,
                                          hb * TB + lo : hb * TB + hi],
                                    ob[:, lo:hi],
                                )
                            else:
                                nc.vector.tensor_copy(out=ob[:, lo:hi],
                                                      in_=po[tt][:, lo:hi])
                                nc.sync.dma_start(
                                    out_d[tt * P : (tt + 1) * P,
                                          hb * TB + lo : hb * TB + hi],
                                    ob[:, lo:hi],
                                )


def build_nc(T=T, H=H, F=F):
    nc = bacc.Bacc(
        "TRN2", target_bir_lowering=False, debug=False, enable_asserts=False
    )
    with tile.TileContext(nc) as tc:
        build_kernel_body(tc, T=T, H=H, F=F)
    nc.compile()
    return nc


_NC_CACHE = None


def run(hidden_states, gate_up_proj, down_proj, trace=False, **kw):
    """Run on the 8 NeuronCores; returns (output, BassKernelResults)."""
    global _NC_CACHE
    if _NC_CACHE is None:
        _NC_CACHE = build_nc()
    nc = _NC_CACHE

    hs = np.ascontiguousarray(np.asarray(hidden_states), dtype=np.float32)
    gup = np.ascontiguousarray(np.asarray(gate_up_proj), dtype=np.float32)
    dp = np.ascontiguousarray(np.asarray(down_proj), dtype=np.float32)
    assert hs.shape == (N_CORES * T, H), hs.shape
    assert gup.shape == (N_CORES, H, 2 * F), gup.shape
    assert dp.shape == (N_CORES, F, H), dp.shape

    in_maps = [
        {
            "hidden_states": np.ascontiguousarray(hs[i * T : (i + 1) * T]),
            "gate_up_proj": np.ascontiguousarray(gup[i]),
            "down_proj": np.ascontiguousarray(dp[i]),
        }
        for i in range(N_CORES)
    ]
    res = run_bass_kernel_spmd(
        nc, in_maps, core_ids=list(range(N_CORES)), trace=trace, **kw
    )
    out = np.concatenate(
        [res.results[i]["out"] for i in range(N_CORES)], axis=0
    ).astype(np.float32)
    return out, res


def kernel(hidden_states, gate_up_proj, down_proj):
    out, _ = run(hidden_states, gate_up_proj, down_proj, trace=False)
    return out


# revision 30
# speedup vs baseline: 1.0100x; 1.0038x over previous
"""Llama4-style MoE experts (grouped SwiGLU MLP) on Trainium2, 8 NeuronCores.

Expert-parallel: core i runs expert i's full MLP on its 1024-token slice:
    out = (up * silu(gate)) @ W2,  [gate|up] = h @ W1
Per-core shapes: h [1024, 2048], W1 [2048, 8192], W2 [4096, 2048].

Matmuls run in bf16 on the TensorEngine.  All weight traffic uses gpsimd
SWDGE *casting* DMAs (f32 HBM -> bf16 SBUF directly), so there is no
f32 staging and no DVE cast pipeline: the PE waits only on DMA-complete
semaphores.  The W1 bf16 stripe ring holds two full 512-wide f-blocks,
so block fb+1 streams in while fb computes with zero boundary stalls.

h loads are split across the sync and vector HWDGE queues (4 rows
each); rows are PE-transposed (f32, via identity) as they land, and
fb0's matmuls run tb-major with i-pair PSUM groups so mm1 starts once
the first 4 rows have landed, with rows 4-7 transposed in the gaps.

mm2 runs f-outer / tt-inner with all 8 PSUM banks accumulating; W2
stripes trickle in ~12 f-tiles ahead of consumption.  Output drains
alternate the Scalar and Vector engines (copy + per-engine DMA queue),
and the last h-block's f-tail is processed per-tt so the drain
staggers into a short tail.
"""

from contextlib import ExitStack

import numpy as np

import concourse.bass as bass
import concourse.mybir as mybir
import concourse.tile as tile
from concourse import bacc
from concourse.bass_utils import run_bass_kernel_spmd
from concourse.masks import make_identity

N_CORES = 8
P = 128
TB = 512  # moving-operand free-dim block (one PSUM bank of f32)

F32 = mybir.dt.float32
BF16 = mybir.dt.bfloat16
ACT_SILU = mybir.ActivationFunctionType.Silu
ACT_COPY = mybir.ActivationFunctionType.Copy

# Per-core problem dims (full problem: 8 experts x 1024 tokens, H=2048, F=4096)
T = 1024
H = 2048
F = 4096


def build_kernel_body(tc, T=T, H=H, F=F):
    nc = tc.nc
    h_d = nc.dram_tensor("hidden_states", [T, H], F32, kind="ExternalInput").ap()
    w1_d = nc.dram_tensor("gate_up_proj", [H, 2 * F], F32, kind="ExternalInput").ap()
    w2_d = nc.dram_tensor("down_proj", [F, H], F32, kind="ExternalInput").ap()
    out_d = nc.dram_tensor("out", [T, H], F32, kind="ExternalOutput").ap()

    n_ht = H // P          # h-tiles (contraction tiles of matmul 1)
    n_ft = F // P          # f-tiles (rows of act; contraction tiles of matmul 2)
    n_tt = T // P          # token tiles (psum partition tiles of matmul 2)
    n_tb = T // TB         # token free-dim blocks in matmul 1
    n_fb = F // TB         # 512-wide f blocks of W1 (per gate/up half)
    n_hb = H // TB         # 512-wide h blocks of W2

    with ExitStack() as ctx:
        const = ctx.enter_context(tc.tile_pool(name="const", bufs=1))
        hbfp = ctx.enter_context(tc.tile_pool(name="hbfp", bufs=4))
        htp = ctx.enter_context(tc.tile_pool(name="htp", bufs=n_ht))
        actp = ctx.enter_context(tc.tile_pool(name="actp", bufs=n_ft))
        w1p = ctx.enter_context(tc.tile_pool(name="w1p", bufs=62))
        b2p = ctx.enter_context(tc.tile_pool(name="b2p", bufs=12))
        silp = ctx.enter_context(tc.tile_pool(name="silp", bufs=2))
        outp = ctx.enter_context(tc.tile_pool(name="outp", bufs=8))
        ps = ctx.enter_context(tc.tile_pool(name="ps", bufs=8, space="PSUM"))

        identb = const.tile([P, P], BF16, tag="identb", name="identb")
        make_identity(nc, identb)

        ht = [htp.tile([P, T], BF16, tag="ht", name=f"ht{i}") for i in range(n_ht)]
        act = [actp.tile([P, T], BF16, tag="act", name=f"act{i}") for i in range(n_ft)]

        # ---- h rows: gpsimd casting DMAs (f32 HBM -> bf16 SBUF) ----
        hrow = {}

        def dma_h_row(ti):
            hr = hbfp.tile([P, H], BF16, tag="hrow", name=f"hrow{ti}")
            nc.gpsimd.dma_start(out=hr[:], in_=h_d[ti * P : (ti + 1) * P, :])
            hrow[ti] = hr

        def transpose_row(ti):
            """PE-transpose one bf16 128-token row block into ht."""
            hr = hrow[ti]
            for hh in range(n_ht):
                pt = ps.tile([P, TB], F32, tag="ps", name=f"tp{ti}_{hh}")
                ptb = pt[:].bitcast(BF16)
                nc.tensor.transpose(
                    ptb[:, :P], hr[:, hh * P : (hh + 1) * P], identb
                )
                nc.vector.tensor_copy(
                    out=ht[hh][:, ti * P : (ti + 1) * P], in_=ptb[:, :P]
                )

        def warmup_pe(n=32):
            """Junk identity matmuls that fill the pre-row0 dead window so
            the PE is at max p-state when the first transpose arrives."""
            for k in range(n):
                wt = ps.tile([P, TB], F32, tag="ps", name=f"wu{k}")
                nc.tensor.matmul(
                    wt[:, :P], lhsT=identb[:], rhs=identb[:],
                    start=True, stop=True,
                )

        # ---- W1: gpsimd casting DMAs, f32 HBM -> bf16 SBUF stripes ----
        w1g, w1u = {}, {}

        def dma_w1_block(fb):
            c0 = fb * TB
            for hh in range(n_ht):
                g = w1p.tile([P, TB], BF16, tag="w1", name=f"w1g_{fb}_{hh}")
                nc.gpsimd.dma_start(
                    out=g[:], in_=w1_d[hh * P : (hh + 1) * P, c0 : c0 + TB]
                )
                u = w1p.tile([P, TB], BF16, tag="w1", name=f"w1u_{fb}_{hh}")
                nc.gpsimd.dma_start(
                    out=u[:], in_=w1_d[hh * P : (hh + 1) * P, F + c0 : F + c0 + TB]
                )
                w1g[(fb, hh)] = g
                w1u[(fb, hh)] = u

        def swiglu(fi, tb, pg, pu):
            sg = silp.tile([P, TB], BF16, tag="silp", name=f"sig{fi}_{tb}")
            nc.scalar.activation(sg[:], pg[:], ACT_SILU)
            nc.vector.tensor_mul(
                out=act[fi][:, tb * TB : (tb + 1) * TB], in0=pu[:], in1=sg[:]
            )

        # ---- W2: gpsimd casting DMAs straight into the b2 ring ----
        b2 = {}

        def w2_stripe(hb, f):
            b = b2p.tile([P, TB], BF16, tag="b2", name=f"b2_{hb}_{f}")
            nc.gpsimd.dma_start(
                out=b[:], in_=w2_d[f * P : (f + 1) * P, hb * TB : (hb + 1) * TB]
            )
            b2[(hb, f)] = b

        # ---- Phase A/B0: h loads + transposes interleaved with fb0 ----
        # gpsimd issue order: rows 0-3, all of fb0, rows 4-7 (the late
        # rows WAR-wait on rows 0-3's transposes, so they must trail fb0
        # to avoid head-of-line blocking the W1 stream).
        for ti in range(4):
            dma_h_row(ti)
        dma_w1_block(0)
        for ti in range(4, 8):
            dma_h_row(ti)
        warmup_pe()
        for ti in range(4):
            transpose_row(ti)
            if ti < 3:
                # fill the wait for the next row's DMA, keeping p-state up
                warmup_pe(6)

        def sweep_fb0(tb, iset):
            """h-outer sweep over one tb-half of fb0 for an i-pair.

            Paces off the W1 DMA stream: matmul (i, hh) waits only on the
            gpsimd casting DMA of stripe (0, hh).
            """
            pg = {i: ps.tile([P, TB], F32, tag="ps", name=f"pg0_{i}_{tb}")
                  for i in iset}
            pu = {i: ps.tile([P, TB], F32, tag="ps", name=f"pu0_{i}_{tb}")
                  for i in iset}
            for hh in range(n_ht):
                first, last = hh == 0, hh == n_ht - 1
                for i in iset:
                    for p, w in ((pg, w1g), (pu, w1u)):
                        nc.tensor.matmul(
                            p[i][:],
                            lhsT=w[(0, hh)][:, i * P : (i + 1) * P],
                            rhs=ht[hh][:, tb * TB : (tb + 1) * TB],
                            start=first,
                            stop=last,
                        )
            for i in iset:
                swiglu(i, tb, pg[i], pu[i])

        # rows 4-7 transpose between sweeps (they land while sweep A runs;
        # no psum accumulation groups are held at these points).  fb1's
        # stripes are issued after sweep A so startup DMA bandwidth goes
        # to h and fb0 first.
        sweep_fb0(0, (0, 1))
        dma_w1_block(1)
        warmup_pe(4)
        transpose_row(4)
        transpose_row(5)
        sweep_fb0(0, (2, 3))
        warmup_pe(4)
        transpose_row(6)
        transpose_row(7)
        sweep_fb0(1, (0, 1))
        sweep_fb0(1, (2, 3))

        # ---- Phase B: steady-state f-blocks 1..n_fb-1 ----
        for fb in range(1, n_fb):
            if fb + 1 < n_fb:
                dma_w1_block(fb + 1)
            for i in range(TB // P):
                fi = fb * (TB // P) + i
                pg = {tb: ps.tile([P, TB], F32, tag="ps", name=f"pg{fi}_{tb}")
                      for tb in range(n_tb)}
                pu = {tb: ps.tile([P, TB], F32, tag="ps", name=f"pu{fi}_{tb}")
                      for tb in range(n_tb)}
                for hh in range(n_ht):
                    first, last = hh == 0, hh == n_ht - 1
                    for p, w in ((pg, w1g), (pu, w1u)):
                        for tb in range(n_tb):
                            nc.tensor.matmul(
                                p[tb][:],
                                lhsT=w[(fb, hh)][:, i * P : (i + 1) * P],
                                rhs=ht[hh][:, tb * TB : (tb + 1) * TB],
                                start=first,
                                stop=last,
                            )
                for tb in range(n_tb):
                    swiglu(fi, tb, pg[tb], pu[tb])
            if fb == n_fb - 2:
                # W2 hb0 prefetch (capped at the b2 ring depth)
                for f in range(12):
                    w2_stripe(0, f)

        # ---- Phase C: out = act @ W2, f-outer / tt-inner, 8 psum banks ----
        def drain(hb, tt, po):
            ob = outp.tile([P, TB], F32, tag="outp", name=f"ob{hb}_{tt}")
            if tt % 2 == 0:
                nc.scalar.activation(ob[:], po[:], ACT_COPY)
                nc.scalar.dma_start(
                    out_d[tt * P : (tt + 1) * P, hb * TB : (hb + 1) * TB], ob[:]
                )
            else:
                nc.vector.tensor_copy(out=ob[:], in_=po[:])
                nc.sync.dma_start(
                    out_d[tt * P : (tt + 1) * P, hb * TB : (hb + 1) * TB], ob[:]
                )

        # strict FIFO stripe schedule: allocation runs exactly AHEAD of
        # consumption so the b2 ring can never form a cross-phase cycle
        SCHED = [(hb, f) for hb in range(n_hb) for f in range(n_ft)]
        AHEAD = 12

        def trickle(g):
            if g + AHEAD < len(SCHED):
                w2_stripe(*SCHED[g + AHEAD])

        FTAIL = 4  # last-hb f-tiles processed per-tt so the drain staggers
        for hb in range(n_hb):
            last_hb = hb == n_hb - 1
            po = [ps.tile([P, TB], F32, tag="ps", name=f"po{hb}_{tt}")
                  for tt in range(n_tt)]
            n_f_main = n_ft - FTAIL if last_hb else n_ft
            for f in range(n_f_main):
                trickle(hb * n_ft + f)
                rhs = b2[(hb, f)][:]
                for tt in range(n_tt):
                    nc.tensor.matmul(
                        po[tt][:],
                        lhsT=act[f][:, tt * P : (tt + 1) * P],
                        rhs=rhs,
                        start=(f == 0),
                        stop=(not last_hb and f == n_ft - 1),
                    )
            if not last_hb:
                for tt in range(n_tt):
                    drain(hb, tt, po[tt])
            else:
                for f in range(n_f_main, n_ft):
                    trickle(hb * n_ft + f)
                for tt in range(n_tt):
                    for f in range(n_f_main, n_ft):
                        nc.tensor.matmul(
                            po[tt][:],
                            lhsT=act[f][:, tt * P : (tt + 1) * P],
                            rhs=b2[(hb, f)][:],
                            start=False,
                            stop=(f == n_ft - 1),
                        )
                    if tt < n_tt - 2:
                        drain(hb, tt, po[tt])
                    else:
                        # final drains split in half across scalar+vector so
                        # the copy and out-DMA tail overlap
                        ob = outp.tile([P, TB], F32, tag="outp",
                                       name=f"ob{hb}_{tt}")
                        hw = TB // 2
                        nc.scalar.activation(ob[:, :hw], po[tt][:, :hw],
                                             ACT_COPY)
                        nc.scalar.dma_start(
                            out_d[tt * P : (tt + 1) * P,
                                  hb * TB : hb * TB + hw],
                            ob[:, :hw],
                        )
                        nc.vector.tensor_copy(out=ob[:, hw:],
                                              in_=po[tt][:, hw:])
                        nc.sync.dma_start(
                            out_d[tt * P : (tt + 1) * P,
                                  hb * TB + hw : (hb + 1) * TB],
                            ob[:, hw:],
                        )


def build_nc(T=T, H=H, F=F):
    nc = bacc.Bacc(
        "TRN2", target_bir_lowering=False, debug=False, enable_asserts=False
    )
    with tile.TileContext(nc) as tc:
        build_kernel_body(tc, T=T, H=H, F=F)
    nc.compile()
    return nc


_NC_CACHE = None


def run(hidden_states, gate_up_proj, down_proj, trace=False, **kw):
    """Run on the 8 NeuronCores; returns (output, BassKernelResults)."""
    global _NC_CACHE
    if _NC_CACHE is None:
        _NC_CACHE = build_nc()
    nc = _NC_CACHE

    hs = np.ascontiguousarray(np.asarray(hidden_states), dtype=np.float32)
    gup = np.ascontiguousarray(np.asarray(gate_up_proj), dtype=np.float32)
    dp = np.ascontiguousarray(np.asarray(down_proj), dtype=np.float32)
    assert hs.shape == (N_CORES * T, H), hs.shape
    assert gup.shape == (N_CORES, H, 2 * F), gup.shape
    assert dp.shape == (N_CORES, F, H), dp.shape

    in_maps = [
        {
            "hidden_states": np.ascontiguousarray(hs[i * T : (i + 1) * T]),
            "gate_up_proj": np.ascontiguousarray(gup[i]),
            "down_proj": np.ascontiguousarray(dp[i]),
        }
        for i in range(N_CORES)
    ]
    res = run_bass_kernel_spmd(
        nc, in_maps, core_ids=list(range(N_CORES)), trace=trace, **kw
    )
    out = np.concatenate(
        [res.results[i]["out"] for i in range(N_CORES)], axis=0
    ).astype(np.float32)
    return out, res


def kernel(hidden_states, gate_up_proj, down_proj):
    out, _ = run(hidden_states, gate_up_proj, down_proj, trace=False)
    return out


# revision 32
# speedup vs baseline: 1.0136x; 1.0036x over previous
"""Llama4-style MoE experts (grouped SwiGLU MLP) on Trainium2, 8 NeuronCores.

Expert-parallel: core i runs expert i's full MLP on its 1024-token slice:
    out = (up * silu(gate)) @ W2,  [gate|up] = h @ W1
Per-core shapes: h [1024, 2048], W1 [2048, 8192], W2 [4096, 2048].

Matmuls run in bf16 on the TensorEngine.  All weight traffic uses gpsimd
SWDGE *casting* DMAs (f32 HBM -> bf16 SBUF directly), so there is no
f32 staging and no DVE cast pipeline: the PE waits only on DMA-complete
semaphores.  The W1 bf16 stripe ring holds two full 512-wide f-blocks,
so block fb+1 streams in while fb computes with zero boundary stalls.

h loads are split across the sync and vector HWDGE queues (4 rows
each); rows are PE-transposed (f32, via identity) as they land, and
fb0's matmuls run tb-major with i-pair PSUM groups so mm1 starts once
the first 4 rows have landed, with rows 4-7 transposed in the gaps.

mm2 runs f-outer / tt-inner with all 8 PSUM banks accumulating; W2
stripes trickle in ~12 f-tiles ahead of consumption.  Output drains
alternate the Scalar and Vector engines (copy + per-engine DMA queue),
and the last h-block's f-tail is processed per-tt so the drain
staggers into a short tail.
"""

from contextlib import ExitStack

import numpy as np

import concourse.bass as bass
import concourse.mybir as mybir
import concourse.tile as tile
from concourse import bacc
from concourse.bass_utils import run_bass_kernel_spmd
from concourse.masks import make_identity

N_CORES = 8
P = 128
TB = 512  # moving-operand free-dim block (one PSUM bank of f32)

F32 = mybir.dt.float32
BF16 = mybir.dt.bfloat16
ACT_SILU = mybir.ActivationFunctionType.Silu
ACT_COPY = mybir.ActivationFunctionType.Copy

# Per-core problem dims (full problem: 8 experts x 1024 tokens, H=2048, F=4096)
T = 1024
H = 2048
F = 4096


def build_kernel_body(tc, T=T, H=H, F=F):
    nc = tc.nc
    h_d = nc.dram_tensor("hidden_states", [T, H], F32, kind="ExternalInput").ap()
    w1_d = nc.dram_tensor("gate_up_proj", [H, 2 * F], F32, kind="ExternalInput").ap()
    w2_d = nc.dram_tensor("down_proj", [F, H], F32, kind="ExternalInput").ap()
    out_d = nc.dram_tensor("out", [T, H], F32, kind="ExternalOutput").ap()

    n_ht = H // P          # h-tiles (contraction tiles of matmul 1)
    n_ft = F // P          # f-tiles (rows of act; contraction tiles of matmul 2)
    n_tt = T // P          # token tiles (psum partition tiles of matmul 2)
    n_tb = T // TB         # token free-dim blocks in matmul 1
    n_fb = F // TB         # 512-wide f blocks of W1 (per gate/up half)
    n_hb = H // TB         # 512-wide h blocks of W2

    with ExitStack() as ctx:
        const = ctx.enter_context(tc.tile_pool(name="const", bufs=1))
        hbfp = ctx.enter_context(tc.tile_pool(name="hbfp", bufs=4))
        htp = ctx.enter_context(tc.tile_pool(name="htp", bufs=n_ht))
        actp = ctx.enter_context(tc.tile_pool(name="actp", bufs=n_ft))
        w1p = ctx.enter_context(tc.tile_pool(name="w1p", bufs=62))
        b2p = ctx.enter_context(tc.tile_pool(name="b2p", bufs=12))
        silp = ctx.enter_context(tc.tile_pool(name="silp", bufs=2))
        outp = ctx.enter_context(tc.tile_pool(name="outp", bufs=8))
        ps = ctx.enter_context(tc.tile_pool(name="ps", bufs=8, space="PSUM"))

        identb = const.tile([P, P], BF16, tag="identb", name="identb")
        make_identity(nc, identb)

        ht = [htp.tile([P, T], BF16, tag="ht", name=f"ht{i}") for i in range(n_ht)]
        act = [actp.tile([P, T], BF16, tag="act", name=f"act{i}") for i in range(n_ft)]

        # ---- h rows: gpsimd casting DMAs (f32 HBM -> bf16 SBUF) ----
        hrow = {}

        def dma_h_row(ti):
            hr = hbfp.tile([P, H], BF16, tag="hrow", name=f"hrow{ti}")
            nc.gpsimd.dma_start(out=hr[:], in_=h_d[ti * P : (ti + 1) * P, :])
            hrow[ti] = hr

        def transpose_row(ti):
            """PE-transpose one bf16 128-token row block into ht."""
            hr = hrow[ti]
            for hh in range(n_ht):
                pt = ps.tile([P, TB], F32, tag="ps", name=f"tp{ti}_{hh}")
                ptb = pt[:].bitcast(BF16)
                nc.tensor.transpose(
                    ptb[:, :P], hr[:, hh * P : (hh + 1) * P], identb
                )
                nc.vector.tensor_copy(
                    out=ht[hh][:, ti * P : (ti + 1) * P], in_=ptb[:, :P]
                )

        def warmup_pe(n=32):
            """Junk identity matmuls that fill the pre-row0 dead window so
            the PE is at max p-state when the first transpose arrives."""
            for k in range(n):
                wt = ps.tile([P, TB], F32, tag="ps", name=f"wu{k}")
                nc.tensor.matmul(
                    wt[:, :P], lhsT=identb[:], rhs=identb[:],
                    start=True, stop=True,
                )

        # ---- W1: gpsimd casting DMAs, f32 HBM -> bf16 SBUF stripes ----
        w1g, w1u = {}, {}

        def dma_w1_block(fb):
            c0 = fb * TB
            for hh in range(n_ht):
                g = w1p.tile([P, TB], BF16, tag="w1", name=f"w1g_{fb}_{hh}")
                nc.gpsimd.dma_start(
                    out=g[:], in_=w1_d[hh * P : (hh + 1) * P, c0 : c0 + TB]
                )
                u = w1p.tile([P, TB], BF16, tag="w1", name=f"w1u_{fb}_{hh}")
                nc.gpsimd.dma_start(
                    out=u[:], in_=w1_d[hh * P : (hh + 1) * P, F + c0 : F + c0 + TB]
                )
                w1g[(fb, hh)] = g
                w1u[(fb, hh)] = u

        def swiglu(fi, tb, pg, pu):
            sg = silp.tile([P, TB], BF16, tag="silp", name=f"sig{fi}_{tb}")
            nc.scalar.activation(sg[:], pg[:], ACT_SILU)
            nc.vector.tensor_mul(
                out=act[fi][:, tb * TB : (tb + 1) * TB], in0=pu[:], in1=sg[:]
            )

        # ---- W2: gpsimd casting DMAs straight into the b2 ring ----
        b2 = {}

        def w2_stripe(hb, f):
            b = b2p.tile([P, TB], BF16, tag="b2", name=f"b2_{hb}_{f}")
            nc.gpsimd.dma_start(
                out=b[:], in_=w2_d[f * P : (f + 1) * P, hb * TB : (hb + 1) * TB]
            )
            b2[(hb, f)] = b

        # ---- Phase A/B0: all h rows first, then fb0 as two full sweeps ----
        # SWDGE order: rows 0-7, then fb0, then fb1.  All transposes run
        # upfront (warmup-filled between row arrivals), so fb0 can run
        # full-width sweeps (both tb) that consume stripe pairs slower
        # (1.72us) than they arrive (~1.4us) -- no mid-sweep stalls.
        for ti in range(8):
            dma_h_row(ti)
        dma_w1_block(0)
        warmup_pe()
        for ti in range(8):
            transpose_row(ti)
            if ti < 7:
                # fill the wait for the next row's DMA, keeping p-state up
                warmup_pe(6)

        def sweep_fb0(iset):
            """h-outer full-width sweep of fb0 for an i-pair (8 psum banks).

            Paces off the W1 DMA stream: matmul (i, hh) waits only on the
            gpsimd casting DMA of stripe (0, hh).
            """
            pg = {(i, tb): ps.tile([P, TB], F32, tag="ps",
                                   name=f"pg0_{i}_{tb}")
                  for i in iset for tb in range(n_tb)}
            pu = {(i, tb): ps.tile([P, TB], F32, tag="ps",
                                   name=f"pu0_{i}_{tb}")
                  for i in iset for tb in range(n_tb)}
            for hh in range(n_ht):
                first, last = hh == 0, hh == n_ht - 1
                for i in iset:
                    for p, w in ((pg, w1g), (pu, w1u)):
                        for tb in range(n_tb):
                            nc.tensor.matmul(
                                p[(i, tb)][:],
                                lhsT=w[(0, hh)][:, i * P : (i + 1) * P],
                                rhs=ht[hh][:, tb * TB : (tb + 1) * TB],
                                start=first,
                                stop=last,
                            )
            for i in iset:
                for tb in range(n_tb):
                    swiglu(i, tb, pg[(i, tb)], pu[(i, tb)])

        sweep_fb0((0, 1))
        dma_w1_block(1)
        sweep_fb0((2, 3))

        # ---- Phase B: steady-state f-blocks 1..n_fb-1 ----
        for fb in range(1, n_fb):
            if fb + 1 < n_fb:
                dma_w1_block(fb + 1)
            for i in range(TB // P):
                fi = fb * (TB // P) + i
                pg = {tb: ps.tile([P, TB], F32, tag="ps", name=f"pg{fi}_{tb}")
                      for tb in range(n_tb)}
                pu = {tb: ps.tile([P, TB], F32, tag="ps", name=f"pu{fi}_{tb}")
                      for tb in range(n_tb)}
                for hh in range(n_ht):
                    first, last = hh == 0, hh == n_ht - 1
                    for p, w in ((pg, w1g), (pu, w1u)):
                        for tb in range(n_tb):
                            nc.tensor.matmul(
                                p[tb][:],
                                lhsT=w[(fb, hh)][:, i * P : (i + 1) * P],
                                rhs=ht[hh][:, tb * TB : (tb + 1) * TB],
                                start=first,
                                stop=last,
                            )
                for tb in range(n_tb):
                    swiglu(fi, tb, pg[tb], pu[tb])
            if fb == n_fb - 2:
                # W2 hb0 prefetch (capped at the b2 ring depth)
                for f in range(12):
                    w2_stripe(0, f)

        # ---- Phase C: out = act @ W2, f-outer / tt-inner, 8 psum banks ----
        def drain(hb, tt, po):
            ob = outp.tile([P, TB], F32, tag="outp", name=f"ob{hb}_{tt}")
            if tt % 2 == 0:
                nc.scalar.activation(ob[:], po[:], ACT_COPY)
                nc.scalar.dma_start(
                    out_d[tt * P : (tt + 1) * P, hb * TB : (hb + 1) * TB], ob[:]
                )
            else:
                nc.vector.tensor_copy(out=ob[:], in_=po[:])
                nc.sync.dma_start(
                    out_d[tt * P : (tt + 1) * P, hb * TB : (hb + 1) * TB], ob[:]
                )

        # strict FIFO stripe schedule: allocation runs exactly AHEAD of
        # consumption so the b2 ring can never form a cross-phase cycle
        SCHED = [(hb, f) for hb in range(n_hb) for f in range(n_ft)]
        AHEAD = 12

        def trickle(g):
            if g + AHEAD < len(SCHED):
                w2_stripe(*SCHED[g + AHEAD])

        FTAIL = 4  # last-hb f-tiles processed per-tt so the drain staggers
        for hb in range(n_hb):
            last_hb = hb == n_hb - 1
            po = [ps.tile([P, TB], F32, tag="ps", name=f"po{hb}_{tt}")
                  for tt in range(n_tt)]
            n_f_main = n_ft - FTAIL if last_hb else n_ft
            for f in range(n_f_main):
                trickle(hb * n_ft + f)
                rhs = b2[(hb, f)][:]
                for tt in range(n_tt):
                    nc.tensor.matmul(
                        po[tt][:],
                        lhsT=act[f][:, tt * P : (tt + 1) * P],
                        rhs=rhs,
                        start=(f == 0),
                        stop=(not last_hb and f == n_ft - 1),
                    )
            if not last_hb:
                for tt in range(n_tt):
                    drain(hb, tt, po[tt])
            else:
                for f in range(n_f_main, n_ft):
                    trickle(hb * n_ft + f)
                for tt in range(n_tt):
                    for f in range(n_f_main, n_ft):
                        nc.tensor.matmul(
                            po[tt][:],
                            lhsT=act[f][:, tt * P : (tt + 1) * P],
                            rhs=b2[(hb, f)][:],
                            start=False,
                            stop=(f == n_ft - 1),
                        )
                    if tt < n_tt - 2:
                        drain(hb, tt, po[tt])
                    else:
                        # final drains split in half across scalar+vector so
                        # the copy and out-DMA tail overlap
                        ob = outp.tile([P, TB], F32, tag="outp",
                                       name=f"ob{hb}_{tt}")
                        hw = TB // 2
                        nc.scalar.activation(ob[:, :hw], po[tt][:, :hw],
                                             ACT_COPY)
                        nc.scalar.dma_start(
                            out_d[tt * P : (tt + 1) * P,
                                  hb * TB : hb * TB + hw],
                            ob[:, :hw],
                        )
                        nc.vector.tensor_copy(out=ob[:, hw:],
                                              in_=po[tt][:, hw:])
                        nc.sync.dma_start(
                            out_d[tt * P : (tt + 1) * P,
                                  hb * TB + hw : (hb + 1) * TB],
                            ob[:, hw:],
                        )


def build_nc(T=T, H=H, F=F):
    nc = bacc.Bacc(
        "TRN2", target_bir_lowering=False, debug=False, enable_asserts=False
    )
    with tile.TileContext(nc) as tc:
        build_kernel_body(tc, T=T, H=H, F=F)
    nc.compile()
    return nc


_NC_CACHE = None


def run(hidden_states, gate_up_proj, down_proj, trace=False, **kw):
    """Run on the 8 NeuronCores; returns (output, BassKernelResults)."""
    global _NC_CACHE
    if _NC_CACHE is None:
        _NC_CACHE = build_nc()
    nc = _NC_CACHE

    hs = np.ascontiguousarray(np.asarray(hidden_states), dtype=np.float32)
    gup = np.ascontiguousarray(np.asarray(gate_up_proj), dtype=np.float32)
    dp = np.ascontiguousarray(np.asarray(down_proj), dtype=np.float32)
    assert hs.shape == (N_CORES * T, H), hs.shape
    assert gup.shape == (N_CORES, H, 2 * F), gup.shape
    assert dp.shape == (N_CORES, F, H), dp.shape

    in_maps = [
        {
            "hidden_states": np.ascontiguousarray(hs[i * T : (i + 1) * T]),
            "gate_up_proj": np.ascontiguousarray(gup[i]),
            "down_proj": np.ascontiguousarray(dp[i]),
        }
        for i in range(N_CORES)
    ]
    res = run_bass_kernel_spmd(
        nc, in_maps, core_ids=list(range(N_CORES)), trace=trace, **kw
    )
    out = np.concatenate(
        [res.results[i]["out"] for i in range(N_CORES)], axis=0
    ).astype(np.float32)
    return out, res


def kernel(hidden_states, gate_up_proj, down_proj):
    out, _ = run(hidden_states, gate_up_proj, down_proj, trace=False)
    return out
